# revision 33
# baseline (speedup 1.0000x reference)
"""Trainium2 Bass kernel for nn_EuclideanLoss.

Math (matches the oracle):
    y_t  = transpose(y, (0, 2, 1))                 # [B, N, D]
    pd   = sqrt(sum((x - y_t)^2, axis=-1))         # [B, N]
    dist = mean(pd, axis=0); dist[1:3] *= 1.5
    loss = mean(dist)

Strategy: data-parallel over batch — each of the 8 NeuronCores takes 4
batches and computes its pair distances pd[b, n] on device; the tiny [B, N]
result is gathered to the host, which finishes mean/scale/mean in float64.

The problem is DMA-bound (16MB of input per core), so both loads are laid
out to produce fully address-sequential HBM descriptors (~370 GB/s measured;
the naive row-strided y load runs at ~200 GB/s and 256B-descriptor x loads
at ~60 GB/s):
  * y[b] ([64, 8192] row-major) loads FLAT into [128, 4096]: partition
    p = 2d + nh holds y[d, nh*4096 : (nh+1)*4096] — pure 16KB-contiguous
    descriptors.  (nh = which half of the batch's n-range)
  * x[b] loads as [128, 2, 32, 64] = (q, nh, c, d) with
    n = nh*4096 + q*32 + c — 32 consecutive rows = 8KB descriptors.
Compute per batch (c-groups of 8 columns):
  PE   32 transposes y_v[:, c, :] ([128, 128]) -> PSUM yT[q, c, 2d+nh],
       aligning y to x's n-to-partition map.
  DVE  diff = x - yT  (yT read through a stride-permuted PSUM view)
  ACT  sq = Square(diff)
  DVE  reduce over d -> d2[p, g, nh, c]
  ACT  pd = Sqrt(d2); DMA out.

Output o[b, p, g, nh, c] = pd[b, nh*4096 + p*32 + g*8 + c]; host undoes it.
"""

import numpy as np

import concourse.bacc as bacc
import concourse.bass as bass
import concourse.mybir as mybir
import concourse.tile as tile
from concourse import masks
from concourse.bass_utils import run_bass_kernel_spmd

B, N, D = 32, 8192, 64
NCORES = 8
BL = B // NCORES        # 4 local batches per core
P = 128                 # SBUF partitions
NH = 2                  # n-halves per batch (partition interleave of y)
CPB = N // NH // P      # 32 consecutive x rows per partition per half
NG = 4                  # c-groups per batch
GC = CPB // NG          # 8 columns per group

F32 = mybir.dt.float32


def _build() -> bass.Bass:
    # Bacc (not plain Bass): its compile() pass splits sem waits across
    # event-semaphore instructions — TRN2 instructions hold at most one wait,
    # and this walrus build rejects multi-wait instructions outright.
    nc = bacc.Bacc("TRN2", target_bir_lowering=False, debug=False, num_devices=NCORES)
    x_d = nc.dram_tensor("x", [BL, N, D], F32, kind="ExternalInput")
    y_d = nc.dram_tensor("y", [BL, D, N], F32, kind="ExternalInput")
    o_d = nc.dram_tensor("o", [P, BL, NG, NH, GC], F32, kind="ExternalOutput")

    with tile.TileContext(nc) as tc:
        with (
            tc.tile_pool(name="const", bufs=1) as cpool,
            tc.tile_pool(name="io", bufs=4) as iopool,
            tc.tile_pool(name="work", bufs=4) as wpool,
            tc.tile_pool(name="res", bufs=2) as rpool,
            tc.tile_pool(name="psum", bufs=4, space="PSUM") as ppool,
        ):
            ident = cpool.tile([P, P], F32)
            masks.make_identity(nc, ident[:])
            d2a = cpool.tile([P, BL, NG, NH, GC], F32)
            # Warm the Sqrt LUT during the DMA fill so the final sqrt does
            # not stall ~1.3us on a lazy ACT_TABLE_LOAD.
            warm = cpool.tile([P, 1], F32)
            nc.scalar.activation(
                warm[:], ident[:, 0:1], mybir.ActivationFunctionType.Sqrt
            )

            for b in range(BL):
                x_t = iopool.tile([P, NH, CPB, D], F32, tag="x")
                y_t = iopool.tile([P, NH * CPB * D], F32, tag="y")
                # y first: the transposes depend only on y, so PE can start
                # while x is still streaming in.  x splits into per-group
                # c-range DMAs so subs can begin before the whole batch lands.
                nc.sync.dma_start(
                    y_t[:], y_d[b].rearrange("d (nh n) -> (d nh) n", nh=NH)
                )
                xsrc = x_d[b].rearrange("(nh q c) d -> q nh c d", nh=NH, c=CPB)
                for g in range(NG):
                    nc.sync.dma_start(
                        x_t[:, :, g * GC : (g + 1) * GC, :],
                        xsrc[:, :, g * GC : (g + 1) * GC, :],
                    )

                # column q of slice c holds n-offset q*32+c within each half
                y_v = y_t[:].rearrange("p (q c) -> p c q", c=CPB)
                for g in range(NG):
                    yT = ppool.tile([P, GC, P], F32, tag="yT")
                    for c in range(GC):
                        nc.tensor.transpose(
                            yT[:, c, :], y_v[:, g * GC + c, :], ident[:]
                        )

                    diff = wpool.tile([P, NH, GC, D], F32, tag="diff")
                    nc.vector.tensor_sub(
                        diff[:],
                        x_t[:, :, g * GC : (g + 1) * GC, :],
                        yT[:].rearrange("p c (d nh) -> p nh c d", nh=NH),
                    )
                    sq = wpool.tile([P, NH, GC, D], F32, tag="sq")
                    nc.scalar.activation(
                        sq[:], diff[:], mybir.ActivationFunctionType.Square
                    )
                    nc.vector.tensor_reduce(
                        d2a[:, b, g, :, :],
                        sq[:],
                        axis=mybir.AxisListType.X,
                        op=mybir.AluOpType.add,
                    )

            # One Sqrt for all batches: avoids per-batch ACT function-table
            # reloads (Square<->Sqrt thrash costs ~1.3us each).
            pda = rpool.tile([P, BL, NG, NH, GC], F32, tag="pd")
            nc.scalar.activation(pda[:], d2a[:], mybir.ActivationFunctionType.Sqrt)
            nc.sync.dma_start(o_d[:], pda[:])
    nc.finalize()
    return nc


_NC_CACHE: list = []


def _get_program() -> bass.Bass:
    if not _NC_CACHE:
        _NC_CACHE.append(_build())
    return _NC_CACHE[0]


def kernel(x: np.ndarray, y: np.ndarray) -> np.ndarray:
    x = np.ascontiguousarray(np.asarray(x, dtype=np.float32))
    y = np.ascontiguousarray(np.asarray(y, dtype=np.float32))
    assert x.shape == (B, N, D) and y.shape == (B, D, N)

    nc = _get_program()
    in_maps = [
        {"x": x[i * BL : (i + 1) * BL], "y": y[i * BL : (i + 1) * BL]}
        for i in range(NCORES)
    ]
    res = run_bass_kernel_spmd(nc, in_maps, list(range(NCORES)))
    o = np.stack([res.results[i]["o"] for i in range(NCORES)])  # [8, P, BL, NG, NH, GC]
    # o[core, p, b, g, nh, c] = pd[core*BL + b, nh*4096 + p*32 + g*8 + c]
    pd = (
        o.transpose(0, 2, 4, 1, 3, 5)  # (core, b, nh, p, g, c)
        .reshape(B, N)
    )

    dist = pd.mean(axis=0, dtype=np.float64)
    dist[1:3] *= 1.5
    return np.asarray(dist.mean(), dtype=np.float32)
